# revision 32
# baseline (speedup 1.0000x reference)
"""Trainium2 Bass kernel for nn_Attention_48309792145474.

Multi-head attention (GQA 32q/8kv heads, head_dim 128, RoPE, causal) for
x:[2,2048,4096], tensor-parallel over heads across 8 NeuronCores.

v2 design (all-bf16 data path, fp32 PSUM accumulation):
  - per core c of 8: q-heads 4c..4c+3, kv-head c; wq/wk/wv column shards,
    wo row shard; host sums the 8 bf16 partial outputs.
  - host pre-transposes x -> xT (bf16) and permutes wq/wk columns so RoPE
    pairs land as [real(0:64); imag(64:128)] halves; 1/sqrt(hd) in wq.
  - phase Q: QKV GEMMs in bf16; RoPE on ACT(copies)+PE(swap)+DVE(muls);
    q/k/v stay SBUF-resident in bf16 (no DRAM roundtrip).
  - phase A: transposed-scores flash attention, bf16 operands:
      scoresT[tk,tq] = kT_tile^T @ qT  -> exp on ACT -> bf16 probs
      causal mask applied as a post-exp binary multiply (DVE 4x mode)
      row sums: bf16 acc (DVE 4x) + ones-matmul; reciprocal on DVE;
      broadcast via SBUF->SBUF DMA; normalize into attn_sb (bf16)
  - phase W: out chunk = attn^T @ wo rows, PSUM->SBUF bf16 on ACT,
    DMA bf16 partial [4096,4096] per core; host sums in fp32.
"""
import os
import sys

os.environ.setdefault("MYCRO_LOCAL_CACHE", "1")

for _p in ("/opt/trn_rl_repo",):
    if os.path.isdir(_p) and _p not in sys.path:
        sys.path.insert(0, _p)

import numpy as np  # noqa: E402
import ml_dtypes  # noqa: E402

import concourse.bass as bass  # noqa: E402
import concourse.mybir as mybir  # noqa: E402
from concourse import bacc, tile  # noqa: E402
from concourse.bass_utils import run_bass_kernel_spmd  # noqa: E402
from concourse.tile_rust import add_dep_helper  # noqa: E402
from contextlib import ExitStack  # noqa: E402

B, T, D = 2, 2048, 4096
H, HKV, HD = 32, 8, 128
BT = B * T
NCORE = 8
QH = H // NCORE          # 4 q-heads per core
CW = 512                 # phase-Q token-chunk width
TCH = BT // CW           # 8 chunks
KBLK = D // 128          # 32 contraction blocks

F32 = mybir.dt.float32
BF16 = mybir.dt.bfloat16
EXP = mybir.ActivationFunctionType.Exp
NPBF = ml_dtypes.bfloat16

LAST_EXEC_NS = None
_CACHE = {}
PHASES = os.environ.get("KPHASES", "qaw")


def _build_nc(repeats=1, phases=None):
    if phases is None:
        phases = PHASES
    nc = bacc.Bacc("TRN2", target_bir_lowering=False, debug=False, num_devices=NCORE)

    # all inputs pre-arranged on host so each DMA reads contiguous per-partition
    # lines (few descriptors) instead of 1KB strided gathers
    xt_d = nc.dram_tensor("xt", [TCH, 4, 128, 8 * CW], BF16, kind="ExternalInput").ap()
    # wq split into 4 load-chunks of 8 kb-blocks for startup pipelining
    wq_d = nc.dram_tensor("wqb", [4, 128, 8 * 512], BF16, kind="ExternalInput").ap()
    wk_d = nc.dram_tensor("wkb", [128, KBLK * 128], BF16, kind="ExternalInput").ap()
    wv_d = nc.dram_tensor("wvb", [128, KBLK * 128], BF16, kind="ExternalInput").ap()
    wo_d = nc.dram_tensor("wob", [4, 128, 4096], BF16, kind="ExternalInput").ap()
    csa_d = nc.dram_tensor("csa", [128, BT], BF16, kind="ExternalInput").ap()
    csb_d = nc.dram_tensor("csb", [128, BT], BF16, kind="ExternalInput").ap()
    mk_d = nc.dram_tensor("maskt", [128, 2048], BF16, kind="ExternalInput").ap()
    id_d = nc.dram_tensor("ident", [128, 128], BF16, kind="ExternalInput").ap()
    sw_d = nc.dram_tensor("swp", [128, 128], BF16, kind="ExternalInput").ap()
    on_d = nc.dram_tensor("onesc", [128, 1], BF16, kind="ExternalInput").ap()
    # contiguous 128KB blocks per (tb, ep); host reassembles
    out_d = nc.dram_tensor("out", [256, 128, 512], BF16, kind="ExternalOutput").ap()
    attn_d = (nc.dram_tensor("attn_in", [128, QH * BT], BF16, kind="ExternalInput").ap()
              if "w" in phases and "a" not in phases else None)

    if phases == "w":
        # standalone W phase: attn loaded from DRAM, no Q/A
        with tile.TileContext(nc) as tc:
            with ExitStack() as S0:
                for rep in range(repeats):
                    with ExitStack() as SAW:
                        p1 = SAW.enter_context(tc.tile_pool(name=f"p1_{rep}", bufs=1))
                        attn_sb = p1.tile([128, QH * BT], BF16)
                        wo_sb = p1.tile([128, 4 * 4096], BF16)
                        nc.sync.dma_start(out=attn_sb, in_=attn_d)
                        nc.sync.dma_start(
                            out=wo_sb.rearrange("p (a m) -> p a m", a=4),
                            in_=wo_d.rearrange("a p m -> p a m"))
                        orp = SAW.enter_context(tc.tile_pool(name="orp", bufs=3))
                        psw_p = SAW.enter_context(
                            tc.tile_pool(name="psw", bufs=3, space="PSUM"))
                        for tb in range(BT // 128):
                            for ep in range(8):
                                psw = psw_p.tile([128, 512], F32, tag="w",
                                                 name=f"psw_{tb}_{ep}")
                                for db in range(4):
                                    nc.tensor.matmul(
                                        psw,
                                        attn_sb[:, db * BT + tb * 128:db * BT + (tb + 1) * 128],
                                        wo_sb[:, db * 4096 + ep * 512:db * 4096 + (ep + 1) * 512],
                                        start=(db == 0), stop=(db == 3))
                                orow = orp.tile([128, 512], BF16, tag="or",
                                                name=f"or_{tb}_{ep}")
                                nc.scalar.copy(orow, psw)
                                nc.sync.dma_start(out=out_d[tb * 8 + ep], in_=orow)
        nc.finalize()
        return nc

    with tile.TileContext(nc) as tc:
        with ExitStack() as S0:
            cons = S0.enter_context(tc.tile_pool(name="cons", bufs=1))
            mk_sb = cons.tile([128, 2048], BF16)
            ones = cons.tile([128, 1], BF16)
            nc.sync.dma_start(out=mk_sb, in_=mk_d)
            nc.sync.dma_start(out=ones, in_=on_d)

            # arep/wrep/awrep: Q built once, the inner A/W phase repeated
            # `repeats` times — slope timing isolates it in steady state
            inner_reps = repeats if phases in ("arep", "wrep", "awrep") else 1
            outer_reps = 1 if phases in ("arep", "wrep", "awrep") else repeats
            for rep in range(outer_reps):
                # persistent per-rep SBUF: q/k/v (bf16), attn, wo
                P0 = tc.tile_pool(name=f"p0_{rep}", bufs=1)
                with P0 as p0:
                    qt_sb = p0.tile([128, QH * BT], BF16)   # per head [128, BT]
                    kt_sb = p0.tile([128, BT], BF16)
                    v_sb = p0.tile([128, 32 * 128], BF16)   # [tk-part, blk*128+d]

                    # ------------------------------------------------ phase Q
                    with ExitStack() as SQ:
                        wp = SQ.enter_context(tc.tile_pool(name="wp", bufs=1))
                        wq_sb = wp.tile([128, KBLK * 512], BF16)
                        wk_sb = wp.tile([128, KBLK * 128], BF16)
                        wv_sb = wp.tile([128, KBLK * 128], BF16)
                        csa_sb = wp.tile([128, BT], BF16)
                        csb_sb = wp.tile([128, BT], BF16)
                        xp = SQ.enter_context(tc.tile_pool(name="xp", bufs=8))
                        stg = SQ.enter_context(tc.tile_pool(name="stg", bufs=3))
                        psq = SQ.enter_context(tc.tile_pool(name="psq", bufs=1, space="PSUM"))

                        def load_xq(ch, only=None):
                            xqs = []
                            for q in range(4):
                                if only is not None and q not in only:
                                    xqs.append(None)
                                    continue
                                xq = xp.tile([128, 8 * CW], BF16, tag="xq", name=f"xq_{ch}_{q}")
                                nc.sync.dma_start(out=xq, in_=xt_d[ch, q])
                                xqs.append(xq)
                            return xqs

                        # startup-ordered weight loads: first halves/chunks
                        # land before chunk-0 x so kb 0 can start ~10us in
                        def wload_q(wc):
                            nc.sync.dma_start(
                                out=wq_sb[:, wc * 4096:(wc + 1) * 4096],
                                in_=wq_d[wc])
                        # round 0 consumes wq immediately; wk/wv only at round 1
                        xq0a = load_xq(0, only={0})
                        wload_q(0)
                        xq0b = load_xq(0, only={1, 2, 3})
                        xq0 = [xq0a[0]] + xq0b[1:]
                        for wc in range(1, 4):
                            wload_q(wc)
                        nc.sync.dma_start(out=wk_sb, in_=wk_d)
                        nc.sync.dma_start(out=wv_sb, in_=wv_d)
                        nc.sync.dma_start(out=csa_sb, in_=csa_d)
                        nc.sync.dma_start(out=csb_sb, in_=csb_d)

                        for ch in range(TCH):
                            xqs = xq0 if ch == 0 else load_xq(ch)

                            # two rounds of 3 full banks each:
                            #   r0 = q0,q1,q2   r1 = q3,k,v
                            banks = []
                            for r, tags in ((0, ("bA", "bB", "bC")),
                                            (1, ("bD", "bE", "bF"))):
                                rb = [psq.tile([128, 512], F32, tag=t,
                                               name=f"{t}_{ch}") for t in tags]
                                banks.append(rb)
                                for kb in range(KBLK):
                                    rhs = xqs[kb // 8][:, (kb % 8) * CW:(kb % 8 + 1) * CW]
                                    st, sp = kb == 0, kb == KBLK - 1
                                    w0 = kb * 512
                                    cols = ([wq_sb[:, w0 + i * 128:w0 + (i + 1) * 128] for i in range(3)]
                                            if r == 0 else
                                            [wq_sb[:, w0 + 384:w0 + 512],
                                             wk_sb[:, kb * 128:(kb + 1) * 128],
                                             wv_sb[:, kb * 128:(kb + 1) * 128]])
                                    for bank, lhs in zip(rb, cols):
                                        nc.tensor.matmul(bank, lhs, rhs, start=st, stop=sp)

                            c0 = ch * CW
                            asl = csa_sb[:, c0:c0 + CW]
                            bsl = csb_sb[:, c0:c0 + CW]

                            def rope_out(ps, dst, name):
                                # z = [r; i] (psum -> sbuf bf16); zs = halves
                                # swapped via SBUF->SBUF DMA (partition move);
                                # out = z*[c;c] + zs*[-s;s]
                                z = stg.tile([128, CW], BF16, tag="z", name=f"z_{name}")
                                nc.scalar.copy(z, ps)
                                zs = stg.tile([128, CW], BF16, tag="zs", name=f"zs_{name}")
                                nc.sync.dma_start(out=zs[64:128, :], in_=z[0:64, :])
                                nc.sync.dma_start(out=zs[0:64, :], in_=z[64:128, :])
                                u = stg.tile([128, CW], BF16, tag="u", name=f"u_{name}")
                                w = stg.tile([128, CW], BF16, tag="w", name=f"w_{name}")
                                nc.vector.tensor_mul(u, z, asl)
                                nc.vector.tensor_mul(w, zs, bsl)
                                nc.vector.tensor_add(dst, u, w)

                            for hh in range(4):
                                ps = banks[hh // 3][hh % 3] if hh < 3 else banks[1][0]
                                rope_out(ps, qt_sb[:, hh * BT + c0:hh * BT + c0 + CW], f"{ch}_{hh}")
                            rope_out(banks[1][1], kt_sb[:, c0:c0 + CW], f"{ch}_k")

                            # v: copy vT psum -> sbuf bf16, DMA-transpose to [t, d]
                            vs = stg.tile([128, CW], BF16, tag="vs", name=f"vs_{ch}")
                            nc.scalar.copy(vs, banks[1][2])
                            for i in range(4):
                                nc.sync.dma_start_transpose(
                                    out=v_sb[:, (4 * ch + i) * 128:(4 * ch + i + 1) * 128],
                                    in_=vs[:, i * 128:(i + 1) * 128])

                    if phases == "q":
                        continue
                    if "m" in phases:
                        # ------------------- merged A + W (single scope) ----
                        # Attention and out-projection interleaved: W matmuls
                        # woven into the scores/PV stream so the PE never
                        # idles on ACT(exp)/DVE chains, and exp/copy work
                        # hides under W matmuls.
                        inv_d = nc.dram_tensor(f"inv_m{rep}", [B * QH * 4, 512], F32).ap()
                        with ExitStack() as SM:
                            p1 = SM.enter_context(tc.tile_pool(name=f"p1_{rep}", bufs=1))
                            attn_sb = p1.tile([128, QH * BT], BF16)
                            wo_sb = p1.tile([128, 4 * 4096], BF16)
                            nc.sync.dma_start(
                                out=wo_sb.rearrange("p (a m) -> p a m", a=4),
                                in_=wo_d.rearrange("a p m -> p a m"))
                            prp = SM.enter_context(tc.tile_pool(name="prp", bufs=4))
                            acp = SM.enter_context(tc.tile_pool(name="acp", bufs=3))
                            ivp = SM.enter_context(tc.tile_pool(name="ivp", bufs=4))
                            orp = SM.enter_context(tc.tile_pool(name="orp", bufs=4))
                            pss_p = SM.enter_context(tc.tile_pool(name="pss", bufs=2, space="PSUM"))
                            pso_p = SM.enter_context(tc.tile_pool(name="pso", bufs=2, space="PSUM"))
                            psm_p = SM.enter_context(tc.tile_pool(name="psm", bufs=1, space="PSUM"))
                            psw_p = SM.enter_context(tc.tile_pool(name="psw", bufs=1, space="PSUM"))

                            wwork = []   # pending W groups (tb, ep)
                            wopen = []   # at most 1 in-flight group [tile, tb, ep, db]
                            wcnt = [0, 0]

                            def emit_w(n):
                                for _ in range(n):
                                    if not wopen and wwork:
                                        tb, ep = wwork.pop(0)
                                        t = psw_p.tile([128, 512], F32, tag="w",
                                                       name=f"psw_{tb}_{ep}")
                                        wopen.append([t, tb, ep, 0])
                                    if not wopen:
                                        return
                                    g = wopen[0]
                                    wcnt[0] += 1
                                    t, tb, ep, db = g
                                    nc.tensor.matmul(
                                        t,
                                        attn_sb[:, db * BT + tb * 128:db * BT + (tb + 1) * 128],
                                        wo_sb[:, db * 4096 + ep * 512:db * 4096 + (ep + 1) * 512],
                                        start=(db == 0), stop=(db == 3))
                                    g[3] += 1
                                    if g[3] == 4:
                                        wopen.clear()
                                        orow = orp.tile([128, 512], BF16, tag="or",
                                                        name=f"or_{tb}_{ep}")
                                        if wcnt[1] % 2 == 0:
                                            nc.vector.tensor_copy(orow, t)
                                        else:
                                            nc.scalar.copy(orow, t)
                                        wcnt[1] += 1
                                        nc.sync.dma_start(out=out_d[tb * 8 + ep], in_=orow)

                            def emit_fin(fin):
                                # deferred one iteration: sums -> recip ->
                                # dram-roundtrip broadcast -> normalize
                                pso_f, acc_f, it_f, col_f, wbatch = fin
                                emit_w(2)
                                psm = psm_p.tile([1, 512], F32, tag="m", name=f"psm_{it_f}")
                                nc.tensor.matmul(psm, ones, acc_f[:, 0:512], start=True, stop=False)
                                emit_w(1)
                                nc.tensor.matmul(psm, ones, acc_f[:, 512:1024], start=False, stop=True)
                                inv_r = ivp.tile([1, 512], F32, tag="ivr", name=f"ivr_{it_f}")
                                nc.vector.reciprocal(inv_r, psm)
                                nc.sync.dma_start(out=inv_d[it_f:it_f + 1, :], in_=inv_r)
                                inv_b = ivp.tile([128, 512], F32, tag="ivb", name=f"ivb_{it_f}")
                                nc.sync.dma_start(
                                    out=inv_b, in_=inv_d[it_f:it_f + 1, :].to_broadcast((128, 512)))
                                nc.vector.tensor_mul(attn_sb[:, col_f:col_f + 512], pso_f, inv_b)
                                if wbatch:
                                    wwork.extend(wbatch)

                            pending_fin = None
                            for b in range(B):
                                for jc in range(4):
                                    for h in range(QH):
                                        it = (b * 4 + jc) * QH + h
                                        qof = h * BT + b * T
                                        npair = 2 * (jc + 1)
                                        qsl = qt_sb[:, qof + jc * 512:qof + (jc + 1) * 512]
                                        pso = pso_p.tile([128, 512], F32, tag="o", name=f"pso_{it}")
                                        acc = acp.tile([128, 1024], BF16, tag="acc", name=f"acc_{it}")

                                        def emit_pv(m, probs, pso=pso, b=b, npair=npair):
                                            for half in range(2):
                                                tb = 2 * m + half
                                                nc.tensor.matmul(
                                                    pso,
                                                    v_sb[:, (b * 16 + tb) * 128:(b * 16 + tb + 1) * 128],
                                                    probs[:, half * 512:(half + 1) * 512],
                                                    start=(tb == 0), stop=(tb == npair * 2 - 1))
                                                emit_w(2)

                                        pend = None
                                        for m in range(npair):
                                            if m == 0 and pending_fin is not None:
                                                emit_fin(pending_fin)
                                                pending_fin = None
                                            pss = pss_p.tile([128, 1024], F32, tag="s",
                                                             name=f"pss_{it}_{m}")
                                            probs = prp.tile([128, 1024], BF16, tag="pr",
                                                             name=f"pr_{it}_{m}")
                                            for half in range(2):
                                                tb = 2 * m + half
                                                nc.tensor.matmul(
                                                    pss[:, half * 512:(half + 1) * 512],
                                                    kt_sb[:, b * T + tb * 128:b * T + (tb + 1) * 128],
                                                    qsl, start=True, stop=True)
                                                emit_w(2)
                                            nc.scalar.activation(probs, pss, EXP)
                                            for half in range(2):
                                                o = 2 * m + half - 4 * jc
                                                if o >= 0:
                                                    nc.vector.tensor_mul(
                                                        probs[:, half * 512:(half + 1) * 512],
                                                        probs[:, half * 512:(half + 1) * 512],
                                                        mk_sb[:, o * 512:(o + 1) * 512])
                                            if m == 0:
                                                nc.vector.tensor_copy(acc, probs)
                                            else:
                                                nc.vector.tensor_add(acc, acc, probs)
                                            if pend is not None:
                                                emit_pv(m - 1, pend)
                                            pend = probs
                                        emit_pv(npair - 1, pend)
                                        wbatch = None
                                        if h == QH - 1:
                                            wbatch = [(b * 16 + jc * 4 + i, ep)
                                                      for i in range(4) for ep in range(8)]
                                        pending_fin = (pso, acc, it, qof + jc * 512, wbatch)
                            emit_fin(pending_fin)
                            emit_w(1 << 20)   # drain remaining W work
                        continue
                    # ---------------------------------------- phases A + W
                    with ExitStack() as SAW:
                        p1 = SAW.enter_context(tc.tile_pool(name=f"p1_{rep}", bufs=1))
                        attn_sb = p1.tile([128, QH * BT], BF16)
                        wo_sb = p1.tile([128, 4 * 4096], BF16)
                        nc.sync.dma_start(
                            out=wo_sb.rearrange("p (a m) -> p a m", a=4),
                            in_=wo_d.rearrange("a p m -> p a m"))

                        if "a" not in phases:   # "qw"/"wrep": attn from DRAM
                            nc.sync.dma_start(out=attn_sb, in_=attn_d)

                        def emit_A(irep):
                            # ---------------------------------- phase A
                            inv_d = nc.dram_tensor(
                                f"inv_i{rep}_{irep}", [B * QH * 4, 512], F32).ap()
                            with ExitStack() as SA:
                                prp = SA.enter_context(tc.tile_pool(name="prp", bufs=5))
                                acp = SA.enter_context(tc.tile_pool(name="acp", bufs=3))
                                ivp = SA.enter_context(tc.tile_pool(name="ivp", bufs=4))
                                pss_p = SA.enter_context(tc.tile_pool(name="pss", bufs=2, space="PSUM"))
                                pso_p = SA.enter_context(tc.tile_pool(name="pso", bufs=3, space="PSUM"))
                                psm_p = SA.enter_context(tc.tile_pool(name="psm", bufs=1, space="PSUM"))

                                def emit_fin(fin):
                                    # sums -> reciprocal -> broadcast -> normalize;
                                    # deferred one iteration so the chain latency
                                    # hides under the next iteration's matmuls
                                    pso, acc, it, col = fin
                                    psm = psm_p.tile([1, 512], F32, tag="m", name=f"psm_{it}")
                                    nc.tensor.matmul(psm, ones, acc[:, 0:512], start=True, stop=False)
                                    nc.tensor.matmul(psm, ones, acc[:, 512:1024], start=False, stop=True)
                                    inv_r = ivp.tile([1, 512], F32, tag="ivr", name=f"ivr_{it}")
                                    nc.vector.reciprocal(inv_r, psm)
                                    nc.sync.dma_start(out=inv_d[it:it + 1, :], in_=inv_r)
                                    inv_b = ivp.tile([128, 512], F32, tag="ivb", name=f"ivb_{it}")
                                    nc.sync.dma_start(
                                        out=inv_b, in_=inv_d[it:it + 1, :].to_broadcast((128, 512)))
                                    nc.vector.tensor_mul(attn_sb[:, col:col + 512], pso, inv_b)

                                pending_fin = None
                                for b in range(B):
                                    for h in range(QH):
                                        qof = h * BT + b * T
                                        for jc in range(4):
                                            it = (b * QH + h) * 4 + jc
                                            npair = 2 * (jc + 1)
                                            pso = pso_p.tile([128, 512], F32, tag="o", name=f"pso_{it}")
                                            acc = acp.tile([128, 1024], BF16, tag="acc", name=f"acc_{it}")

                                            def emit_pv(m, probs, pso=pso, b=b, npair=npair):
                                                for half in range(2):
                                                    tb = 2 * m + half
                                                    nc.tensor.matmul(
                                                        pso,
                                                        v_sb[:, (b * 16 + tb) * 128:(b * 16 + tb + 1) * 128],
                                                        probs[:, half * 512:(half + 1) * 512],
                                                        start=(tb == 0), stop=(tb == npair * 2 - 1))

                                            pend = None
                                            for m in range(npair):
                                                pss = pss_p.tile([128, 1024], F32, tag="s", name=f"pss_{it}_{m}")
                                                for half in range(2):
                                                    tb = 2 * m + half
                                                    nc.tensor.matmul(
                                                        pss[:, half * 512:(half + 1) * 512],
                                                        kt_sb[:, b * T + tb * 128:b * T + (tb + 1) * 128],
                                                        qt_sb[:, qof + jc * 512:qof + (jc + 1) * 512],
                                                        start=True, stop=True)
                                                probs = prp.tile([128, 1024], BF16, tag="pr", name=f"pr_{it}_{m}")
                                                nc.scalar.activation(probs, pss, EXP)
                                                for half in range(2):
                                                    o = 2 * m + half - 4 * jc
                                                    if o >= 0:
                                                        nc.vector.tensor_mul(
                                                            probs[:, half * 512:(half + 1) * 512],
                                                            probs[:, half * 512:(half + 1) * 512],
                                                            mk_sb[:, o * 512:(o + 1) * 512])
                                                if m == 0:
                                                    nc.vector.tensor_copy(acc, probs)
                                                else:
                                                    nc.vector.tensor_add(acc, acc, probs)
                                                # PV runs one m behind so exp latency
                                                # hides under the next scores matmuls
                                                if pend is not None:
                                                    emit_pv(m - 1, pend)
                                                pend = probs
                                                if m == 0 and pending_fin is not None:
                                                    emit_fin(pending_fin)
                                                    pending_fin = None
                                            emit_pv(npair - 1, pend)
                                            pending_fin = (pso, acc, it, qof + jc * 512)
                                emit_fin(pending_fin)

                        def emit_W(irep):
                            # ---------------------------------- phase W
                            with ExitStack() as SW:
                                orp = SW.enter_context(tc.tile_pool(name="orp", bufs=4))
                                psw_p = SW.enter_context(tc.tile_pool(name="psw", bufs=4, space="PSUM"))
                                for tb in range(BT // 128):
                                    for epp in range(4):
                                        ep0, ep1 = 2 * epp, 2 * epp + 1
                                        pws = [psw_p.tile([128, 512], F32, tag="w",
                                                          name=f"psw_{irep}_{tb}_{ep}")
                                               for ep in (ep0, ep1)]
                                        for db in range(4):
                                            for pi, ep in ((0, ep0), (1, ep1)):
                                                nc.tensor.matmul(
                                                    pws[pi],
                                                    attn_sb[:, db * BT + tb * 128:db * BT + (tb + 1) * 128],
                                                    wo_sb[:, db * 4096 + ep * 512:db * 4096 + (ep + 1) * 512],
                                                    start=(db == 0), stop=(db == 3))
                                        if "nocopy" in phases:
                                            continue
                                        for pi, ep in ((0, ep0), (1, ep1)):
                                            orow = orp.tile([128, 512], BF16, tag="or",
                                                            name=f"or_{irep}_{tb}_{ep}")
                                            nc.scalar.copy(orow, pws[pi])
                                            if "nodma" in phases:
                                                continue
                                            nc.sync.dma_start(out=out_d[tb * 8 + ep],
                                                              in_=orow)

                        for irep in range(inner_reps):
                            if "a" in phases:
                                emit_A(irep)
                            if phases in ("qa", "arep"):
                                continue
                            emit_W(irep)
    nc.finalize()
    return nc


def _host_prep(x, freqs, wq, wk, wv, wo, mask=None):
    if mask is None:
        ii = np.arange(T)[:, None]
        jj = np.arange(T)[None, :]
        mask = np.where(jj <= ii, np.float32(0.0), np.float32(-1e9))
    _kernel_mask = np.asarray(mask, dtype=np.float32)

    x = np.ascontiguousarray(np.asarray(x, dtype=np.float32).reshape(BT, D))
    xT = x.T.astype(NPBF)                                  # [D, BT] bf16
    # [TCH, 4, 128, 8*CW]: partition p of load (ch, q) reads one contiguous
    # 8KB line = rows {q*1024 + a*128 + p} of xT at chunk ch's columns
    xtc = np.ascontiguousarray(
        xT.reshape(4, 8, 128, TCH, CW).transpose(3, 0, 2, 1, 4)
        .reshape(TCH, 4, 128, 8 * CW))

    freqs = np.asarray(freqs, dtype=np.float32)
    cos = np.tile(np.cos(freqs).T, (1, B))                 # [64, BT]
    sin = np.tile(np.sin(freqs).T, (1, B))
    csa = np.concatenate([cos, cos], axis=0).astype(NPBF)
    csb = np.concatenate([-sin, sin], axis=0).astype(NPBF)
    swp = np.zeros((128, 128), np.float32)
    swp[np.arange(64), 64 + np.arange(64)] = 1.0
    swp[64 + np.arange(64), np.arange(64)] = 1.0

    perm = np.concatenate([np.arange(0, HD, 2), np.arange(1, HD, 2)])
    wq_p = (np.asarray(wq, dtype=np.float32).reshape(D, H, HD)[:, :, perm]
            .reshape(D, H * HD) / np.float32(np.sqrt(HD)))
    wk_p = np.asarray(wk, dtype=np.float32).reshape(D, HKV, HD)[:, :, perm].reshape(D, HKV * HD)
    wv = np.asarray(wv, dtype=np.float32)
    wo = np.asarray(wo, dtype=np.float32)

    # binary mask, transposed band layout:
    # maskt[:, o*512:(o+1)*512][i, j] = 1 if key (128*o+i) visible to query j
    maskt = np.concatenate(
        [(_kernel_mask[0:512, 128 * o:128 * o + 128] > -1.0).T.astype(np.float32)
         for o in range(4)],
        axis=1).astype(NPBF)                               # [128, 2048]
    ident = np.eye(128, dtype=np.float32).astype(NPBF)

    in_maps = []
    for c in range(NCORE):
        wq_c = np.ascontiguousarray(
            wq_p[:, c * 512:(c + 1) * 512].reshape(4, 8, 128, 512)
            .transpose(0, 2, 1, 3).reshape(4, 128, 8 * 512)).astype(NPBF)
        wk_c = np.ascontiguousarray(
            wk_p[:, c * HD:(c + 1) * HD].reshape(KBLK, 128, 128)
            .transpose(1, 0, 2).reshape(128, KBLK * 128)).astype(NPBF)
        wv_c = np.ascontiguousarray(
            wv[:, c * HD:(c + 1) * HD].reshape(KBLK, 128, 128)
            .transpose(1, 0, 2).reshape(128, KBLK * 128)).astype(NPBF)
        wo_c = np.ascontiguousarray(
            wo[c * 512:(c + 1) * 512, :].reshape(4, 128, 4096)).astype(NPBF)
        in_maps.append({
            "xt": xtc, "wqb": wq_c, "wkb": wk_c, "wvb": wv_c, "wob": wo_c,
            "csa": csa, "csb": csb, "maskt": maskt, "ident": ident,
            "swp": swp.astype(NPBF), "onesc": np.ones((128, 1), NPBF),
            "attn_in": np.zeros((128, QH * BT), NPBF),
        })
    return in_maps


def kernel(x, freqs, mask, wq, wk, wv, wo, start_pos=0, **_kw):
    global LAST_EXEC_NS
    in_maps = _host_prep(x, freqs, wq, wk, wv, wo, mask=mask)
    if "nc" not in _CACHE:
        _CACHE["nc"] = _build_nc(phases=PHASES)
    nc = _CACHE["nc"]
    res = run_bass_kernel_spmd(nc, in_maps, core_ids=list(range(NCORE)), trace=False)
    LAST_EXEC_NS = getattr(res, "exec_time_ns", None)
    total = res.results[0]["out"].astype(np.float32)
    for c in range(1, NCORE):
        total = total + res.results[c]["out"].astype(np.float32)
    # out[tb*8+ep, r, c] -> full[tb*128+r, ep*512+c]
    total = (total.reshape(32, 8, 128, 512).transpose(0, 2, 1, 3)
             .reshape(BT, D))
    return total.reshape(B, T, D)



# revision 35
# speedup vs baseline: 1.0364x; 1.0364x over previous
"""Trainium2 Bass kernel for nn_Attention_48309792145474.

Multi-head attention (GQA 32q/8kv heads, head_dim 128, RoPE, causal) for
x:[2,2048,4096], tensor-parallel over heads across 8 NeuronCores.

v2 design (all-bf16 data path, fp32 PSUM accumulation):
  - per core c of 8: q-heads 4c..4c+3, kv-head c; wq/wk/wv column shards,
    wo row shard; host sums the 8 bf16 partial outputs.
  - host pre-transposes x -> xT (bf16) and permutes wq/wk columns so RoPE
    pairs land as [real(0:64); imag(64:128)] halves; 1/sqrt(hd) in wq.
  - phase Q: QKV GEMMs in bf16; RoPE on ACT(copies)+PE(swap)+DVE(muls);
    q/k/v stay SBUF-resident in bf16 (no DRAM roundtrip).
  - phase A: transposed-scores flash attention, bf16 operands:
      scoresT[tk,tq] = kT_tile^T @ qT  -> exp on ACT -> bf16 probs
      causal mask applied as a post-exp binary multiply (DVE 4x mode)
      row sums: bf16 acc (DVE 4x) + ones-matmul; reciprocal on DVE;
      broadcast via SBUF->SBUF DMA; normalize into attn_sb (bf16)
  - phase W: out chunk = attn^T @ wo rows, PSUM->SBUF bf16 on ACT,
    DMA bf16 partial [4096,4096] per core; host sums in fp32.
"""
import os
import sys

os.environ.setdefault("MYCRO_LOCAL_CACHE", "1")

for _p in ("/opt/trn_rl_repo",):
    if os.path.isdir(_p) and _p not in sys.path:
        sys.path.insert(0, _p)

import numpy as np  # noqa: E402
import ml_dtypes  # noqa: E402

import concourse.bass as bass  # noqa: E402
import concourse.mybir as mybir  # noqa: E402
from concourse import bacc, tile  # noqa: E402
from concourse.bass_utils import run_bass_kernel_spmd  # noqa: E402
from concourse.tile_rust import add_dep_helper  # noqa: E402
from contextlib import ExitStack  # noqa: E402

B, T, D = 2, 2048, 4096
H, HKV, HD = 32, 8, 128
BT = B * T
NCORE = 8
QH = H // NCORE          # 4 q-heads per core
CW = 512                 # phase-Q token-chunk width
TCH = BT // CW           # 8 chunks
KBLK = D // 128          # 32 contraction blocks

F32 = mybir.dt.float32
BF16 = mybir.dt.bfloat16
EXP = mybir.ActivationFunctionType.Exp
NPBF = ml_dtypes.bfloat16

LAST_EXEC_NS = None
_CACHE = {}
PHASES = os.environ.get("KPHASES", "qaw")


def _build_nc(repeats=1, phases=None):
    if phases is None:
        phases = PHASES
    nc = bacc.Bacc("TRN2", target_bir_lowering=False, debug=False, num_devices=NCORE)

    # all inputs pre-arranged on host so each DMA reads contiguous per-partition
    # lines (few descriptors) instead of 1KB strided gathers
    xt_d = nc.dram_tensor("xt", [TCH, 4, 128, 8 * CW], BF16, kind="ExternalInput").ap()
    # wq split into 4 load-chunks of 8 kb-blocks for startup pipelining
    wq_d = nc.dram_tensor("wqb", [4, 128, 8 * 512], BF16, kind="ExternalInput").ap()
    wk_d = nc.dram_tensor("wkb", [128, KBLK * 128], BF16, kind="ExternalInput").ap()
    wv_d = nc.dram_tensor("wvb", [128, KBLK * 128], BF16, kind="ExternalInput").ap()
    wo_d = nc.dram_tensor("wob", [4, 128, 4096], BF16, kind="ExternalInput").ap()
    csa_d = nc.dram_tensor("csa", [128, BT], BF16, kind="ExternalInput").ap()
    csb_d = nc.dram_tensor("csb", [128, BT], BF16, kind="ExternalInput").ap()
    mk_d = nc.dram_tensor("maskt", [128, 2048], BF16, kind="ExternalInput").ap()
    id_d = nc.dram_tensor("ident", [128, 128], BF16, kind="ExternalInput").ap()
    sw_d = nc.dram_tensor("swp", [128, 128], BF16, kind="ExternalInput").ap()
    on_d = nc.dram_tensor("onesc", [128, 1], BF16, kind="ExternalInput").ap()
    # contiguous 128KB blocks per (tb, ep); host reassembles
    out_d = nc.dram_tensor("out", [256, 128, 512], BF16, kind="ExternalOutput").ap()
    attn_d = (nc.dram_tensor("attn_in", [128, QH * BT], BF16, kind="ExternalInput").ap()
              if "w" in phases and "a" not in phases else None)

    if phases == "w":
        # standalone W phase: attn loaded from DRAM, no Q/A
        with tile.TileContext(nc) as tc:
            with ExitStack() as S0:
                for rep in range(repeats):
                    with ExitStack() as SAW:
                        p1 = SAW.enter_context(tc.tile_pool(name=f"p1_{rep}", bufs=1))
                        attn_sb = p1.tile([128, QH * BT], BF16)
                        wo_sb = p1.tile([128, 4 * 4096], BF16)
                        nc.sync.dma_start(out=attn_sb, in_=attn_d)
                        nc.sync.dma_start(
                            out=wo_sb.rearrange("p (a m) -> p a m", a=4),
                            in_=wo_d.rearrange("a p m -> p a m"))
                        orp = SAW.enter_context(tc.tile_pool(name="orp", bufs=3))
                        psw_p = SAW.enter_context(
                            tc.tile_pool(name="psw", bufs=3, space="PSUM"))
                        for tb in range(BT // 128):
                            for ep in range(8):
                                psw = psw_p.tile([128, 512], F32, tag="w",
                                                 name=f"psw_{tb}_{ep}")
                                for db in range(4):
                                    nc.tensor.matmul(
                                        psw,
                                        attn_sb[:, db * BT + tb * 128:db * BT + (tb + 1) * 128],
                                        wo_sb[:, db * 4096 + ep * 512:db * 4096 + (ep + 1) * 512],
                                        start=(db == 0), stop=(db == 3))
                                orow = orp.tile([128, 512], BF16, tag="or",
                                                name=f"or_{tb}_{ep}")
                                nc.scalar.copy(orow, psw)
                                nc.sync.dma_start(out=out_d[tb * 8 + ep], in_=orow)
        nc.finalize()
        return nc

    with tile.TileContext(nc) as tc:
        with ExitStack() as S0:
            cons = S0.enter_context(tc.tile_pool(name="cons", bufs=1))
            mk_sb = cons.tile([128, 2048], BF16)
            ones = cons.tile([128, 1], BF16)
            nc.sync.dma_start(out=mk_sb, in_=mk_d)
            nc.sync.dma_start(out=ones, in_=on_d)

            # arep/wrep/awrep: Q built once, the inner A/W phase repeated
            # `repeats` times — slope timing isolates it in steady state
            inner_reps = repeats if phases in ("arep", "wrep", "awrep") else 1
            outer_reps = 1 if phases in ("arep", "wrep", "awrep") else repeats
            for rep in range(outer_reps):
                # persistent per-rep SBUF: q/k/v (bf16), attn, wo
                P0 = tc.tile_pool(name=f"p0_{rep}", bufs=1)
                with P0 as p0:
                    qt_sb = p0.tile([128, QH * BT], BF16)   # per head [128, BT]
                    kt_sb = p0.tile([128, BT], BF16)
                    v_sb = p0.tile([128, 32 * 128], BF16)   # [tk-part, blk*128+d]

                    # ------------------------------------------------ phase Q
                    with ExitStack() as SQ:
                        # SBUF tile creation order is engineered so that the
                        # tiles needed FIRST in the next repeat (x chunk 0,
                        # wq chunk 0) land at addresses that are free during
                        # the previous repeat's A/W phases — their loads then
                        # prefetch across the repeat boundary instead of
                        # starting cold behind the out-DMA drain.
                        wp = SQ.enter_context(tc.tile_pool(name="wp", bufs=1))
                        xp = SQ.enter_context(tc.tile_pool(name="xp", bufs=8))
                        stg = SQ.enter_context(tc.tile_pool(name="stg", bufs=3))
                        psq = SQ.enter_context(tc.tile_pool(name="psq", bufs=1, space="PSUM"))
                        wq123_sb = wp.tile([128, 24 * 512], BF16)   # kb 8..31
                        csa_sb = wp.tile([128, BT], BF16)
                        csb_sb = wp.tile([128, BT], BF16)
                        wk_sb = wp.tile([128, KBLK * 128], BF16)
                        wv_sb = wp.tile([128, KBLK * 128], BF16)
                        for _t, _w in (("z", CW), ("zs", CW), ("u", CW),
                                       ("w", CW), ("vs", CW)):
                            for _i in range(3):   # materialize stg slots here
                                stg.tile([128, _w], BF16, tag=_t, name=f"wu_{_t}{_i}")
                        for _i in range(8):       # materialize xp ring slots
                            xp.tile([128, 8 * CW], BF16, tag="xq", name=f"wu_xq{_i}")
                        wq0_sb = wp.tile([128, 8 * 512], BF16)      # kb 0..7

                        def wq_sl(kb, i0, i1):
                            if kb < 8:
                                return wq0_sb[:, kb * 512 + i0 * 128:kb * 512 + i1 * 128]
                            w0 = (kb - 8) * 512
                            return wq123_sb[:, w0 + i0 * 128:w0 + i1 * 128]

                        def load_xq(ch, only=None):
                            xqs = []
                            for q in range(4):
                                if only is not None and q not in only:
                                    xqs.append(None)
                                    continue
                                xq = xp.tile([128, 8 * CW], BF16, tag="xq", name=f"xq_{ch}_{q}")
                                nc.sync.dma_start(out=xq, in_=xt_d[ch, q])
                                xqs.append(xq)
                            return xqs

                        # startup-ordered weight loads: first halves/chunks
                        # land before chunk-0 x so kb 0 can start ~10us in
                        def wload_q(wc):
                            if wc == 0:
                                nc.sync.dma_start(out=wq0_sb, in_=wq_d[0])
                            else:
                                nc.sync.dma_start(
                                    out=wq123_sb[:, (wc - 1) * 4096:wc * 4096],
                                    in_=wq_d[wc])
                        # round 0 consumes wq immediately; wk/wv only at round 1
                        xq0a = load_xq(0, only={0})
                        wload_q(0)
                        xq0b = load_xq(0, only={1, 2, 3})
                        xq0 = [xq0a[0]] + xq0b[1:]
                        for wc in range(1, 4):
                            wload_q(wc)
                        nc.sync.dma_start(out=wk_sb, in_=wk_d)
                        nc.sync.dma_start(out=wv_sb, in_=wv_d)
                        nc.sync.dma_start(out=csa_sb, in_=csa_d)
                        nc.sync.dma_start(out=csb_sb, in_=csb_d)

                        for ch in range(TCH):
                            xqs = xq0 if ch == 0 else load_xq(ch)

                            # two rounds of 3 full banks each:
                            #   r0 = q0,q1,q2   r1 = q3,k,v
                            banks = []
                            for r, tags in ((0, ("bA", "bB", "bC")),
                                            (1, ("bD", "bE", "bF"))):
                                rb = [psq.tile([128, 512], F32, tag=t,
                                               name=f"{t}_{ch}") for t in tags]
                                banks.append(rb)
                                for kb in range(KBLK):
                                    rhs = xqs[kb // 8][:, (kb % 8) * CW:(kb % 8 + 1) * CW]
                                    st, sp = kb == 0, kb == KBLK - 1
                                    cols = ([wq_sl(kb, i, i + 1) for i in range(3)]
                                            if r == 0 else
                                            [wq_sl(kb, 3, 4),
                                             wk_sb[:, kb * 128:(kb + 1) * 128],
                                             wv_sb[:, kb * 128:(kb + 1) * 128]])
                                    for bank, lhs in zip(rb, cols):
                                        nc.tensor.matmul(bank, lhs, rhs, start=st, stop=sp)

                            c0 = ch * CW
                            asl = csa_sb[:, c0:c0 + CW]
                            bsl = csb_sb[:, c0:c0 + CW]

                            def rope_out(ps, dst, name):
                                # z = [r; i] (psum -> sbuf bf16); zs = halves
                                # swapped via SBUF->SBUF DMA (partition move);
                                # out = z*[c;c] + zs*[-s;s]
                                z = stg.tile([128, CW], BF16, tag="z", name=f"z_{name}")
                                nc.scalar.copy(z, ps)
                                zs = stg.tile([128, CW], BF16, tag="zs", name=f"zs_{name}")
                                nc.sync.dma_start(out=zs[64:128, :], in_=z[0:64, :])
                                nc.sync.dma_start(out=zs[0:64, :], in_=z[64:128, :])
                                u = stg.tile([128, CW], BF16, tag="u", name=f"u_{name}")
                                w = stg.tile([128, CW], BF16, tag="w", name=f"w_{name}")
                                nc.vector.tensor_mul(u, z, asl)
                                nc.vector.tensor_mul(w, zs, bsl)
                                nc.vector.tensor_add(dst, u, w)

                            for hh in range(4):
                                ps = banks[hh // 3][hh % 3] if hh < 3 else banks[1][0]
                                rope_out(ps, qt_sb[:, hh * BT + c0:hh * BT + c0 + CW], f"{ch}_{hh}")
                            rope_out(banks[1][1], kt_sb[:, c0:c0 + CW], f"{ch}_k")

                            # v: copy vT psum -> sbuf bf16, DMA-transpose to [t, d]
                            vs = stg.tile([128, CW], BF16, tag="vs", name=f"vs_{ch}")
                            nc.scalar.copy(vs, banks[1][2])
                            for i in range(4):
                                nc.sync.dma_start_transpose(
                                    out=v_sb[:, (4 * ch + i) * 128:(4 * ch + i + 1) * 128],
                                    in_=vs[:, i * 128:(i + 1) * 128])

                    if phases == "q":
                        continue
                    if "m" in phases:
                        # ------------------- merged A + W (single scope) ----
                        # Attention and out-projection interleaved: W matmuls
                        # woven into the scores/PV stream so the PE never
                        # idles on ACT(exp)/DVE chains, and exp/copy work
                        # hides under W matmuls.
                        inv_d = nc.dram_tensor(f"inv_m{rep}", [B * QH * 4, 512], F32).ap()
                        with ExitStack() as SM:
                            p1 = SM.enter_context(tc.tile_pool(name=f"p1_{rep}", bufs=1))
                            attn_sb = p1.tile([128, QH * BT], BF16)
                            wo_sb = p1.tile([128, 4 * 4096], BF16)
                            nc.sync.dma_start(
                                out=wo_sb.rearrange("p (a m) -> p a m", a=4),
                                in_=wo_d.rearrange("a p m -> p a m"))
                            prp = SM.enter_context(tc.tile_pool(name="prp", bufs=4))
                            acp = SM.enter_context(tc.tile_pool(name="acp", bufs=3))
                            ivp = SM.enter_context(tc.tile_pool(name="ivp", bufs=4))
                            orp = SM.enter_context(tc.tile_pool(name="orp", bufs=4))
                            pss_p = SM.enter_context(tc.tile_pool(name="pss", bufs=2, space="PSUM"))
                            pso_p = SM.enter_context(tc.tile_pool(name="pso", bufs=2, space="PSUM"))
                            psm_p = SM.enter_context(tc.tile_pool(name="psm", bufs=1, space="PSUM"))
                            psw_p = SM.enter_context(tc.tile_pool(name="psw", bufs=1, space="PSUM"))

                            wwork = []   # pending W groups (tb, ep)
                            wopen = []   # at most 1 in-flight group [tile, tb, ep, db]
                            wcnt = [0, 0]

                            def emit_w(n):
                                for _ in range(n):
                                    if not wopen and wwork:
                                        tb, ep = wwork.pop(0)
                                        t = psw_p.tile([128, 512], F32, tag="w",
                                                       name=f"psw_{tb}_{ep}")
                                        wopen.append([t, tb, ep, 0])
                                    if not wopen:
                                        return
                                    g = wopen[0]
                                    wcnt[0] += 1
                                    t, tb, ep, db = g
                                    nc.tensor.matmul(
                                        t,
                                        attn_sb[:, db * BT + tb * 128:db * BT + (tb + 1) * 128],
                                        wo_sb[:, db * 4096 + ep * 512:db * 4096 + (ep + 1) * 512],
                                        start=(db == 0), stop=(db == 3))
                                    g[3] += 1
                                    if g[3] == 4:
                                        wopen.clear()
                                        orow = orp.tile([128, 512], BF16, tag="or",
                                                        name=f"or_{tb}_{ep}")
                                        if wcnt[1] % 2 == 0:
                                            nc.vector.tensor_copy(orow, t)
                                        else:
                                            nc.scalar.copy(orow, t)
                                        wcnt[1] += 1
                                        nc.sync.dma_start(out=out_d[tb * 8 + ep], in_=orow)

                            def emit_fin(fin):
                                # deferred one iteration: sums -> recip ->
                                # dram-roundtrip broadcast -> normalize
                                pso_f, acc_f, it_f, col_f, wbatch = fin
                                emit_w(2)
                                psm = psm_p.tile([1, 512], F32, tag="m", name=f"psm_{it_f}")
                                nc.tensor.matmul(psm, ones, acc_f[:, 0:512], start=True, stop=False)
                                emit_w(1)
                                nc.tensor.matmul(psm, ones, acc_f[:, 512:1024], start=False, stop=True)
                                inv_r = ivp.tile([1, 512], F32, tag="ivr", name=f"ivr_{it_f}")
                                nc.vector.reciprocal(inv_r, psm)
                                nc.sync.dma_start(out=inv_d[it_f:it_f + 1, :], in_=inv_r)
                                inv_b = ivp.tile([128, 512], F32, tag="ivb", name=f"ivb_{it_f}")
                                nc.sync.dma_start(
                                    out=inv_b, in_=inv_d[it_f:it_f + 1, :].to_broadcast((128, 512)))
                                nc.vector.tensor_mul(attn_sb[:, col_f:col_f + 512], pso_f, inv_b)
                                if wbatch:
                                    wwork.extend(wbatch)

                            pending_fin = None
                            for b in range(B):
                                for jc in range(4):
                                    for h in range(QH):
                                        it = (b * 4 + jc) * QH + h
                                        qof = h * BT + b * T
                                        npair = 2 * (jc + 1)
                                        qsl = qt_sb[:, qof + jc * 512:qof + (jc + 1) * 512]
                                        pso = pso_p.tile([128, 512], F32, tag="o", name=f"pso_{it}")
                                        acc = acp.tile([128, 1024], BF16, tag="acc", name=f"acc_{it}")

                                        def emit_pv(m, probs, pso=pso, b=b, npair=npair):
                                            for half in range(2):
                                                tb = 2 * m + half
                                                nc.tensor.matmul(
                                                    pso,
                                                    v_sb[:, (b * 16 + tb) * 128:(b * 16 + tb + 1) * 128],
                                                    probs[:, half * 512:(half + 1) * 512],
                                                    start=(tb == 0), stop=(tb == npair * 2 - 1))
                                                emit_w(2)

                                        pend = None
                                        for m in range(npair):
                                            if m == 0 and pending_fin is not None:
                                                emit_fin(pending_fin)
                                                pending_fin = None
                                            pss = pss_p.tile([128, 1024], F32, tag="s",
                                                             name=f"pss_{it}_{m}")
                                            probs = prp.tile([128, 1024], BF16, tag="pr",
                                                             name=f"pr_{it}_{m}")
                                            for half in range(2):
                                                tb = 2 * m + half
                                                nc.tensor.matmul(
                                                    pss[:, half * 512:(half + 1) * 512],
                                                    kt_sb[:, b * T + tb * 128:b * T + (tb + 1) * 128],
                                                    qsl, start=True, stop=True)
                                                emit_w(2)
                                            nc.scalar.activation(probs, pss, EXP)
                                            for half in range(2):
                                                o = 2 * m + half - 4 * jc
                                                if o >= 0:
                                                    nc.vector.tensor_mul(
                                                        probs[:, half * 512:(half + 1) * 512],
                                                        probs[:, half * 512:(half + 1) * 512],
                                                        mk_sb[:, o * 512:(o + 1) * 512])
                                            if m == 0:
                                                nc.vector.tensor_copy(acc, probs)
                                            else:
                                                nc.vector.tensor_add(acc, acc, probs)
                                            if pend is not None:
                                                emit_pv(m - 1, pend)
                                            pend = probs
                                        emit_pv(npair - 1, pend)
                                        wbatch = None
                                        if h == QH - 1:
                                            wbatch = [(b * 16 + jc * 4 + i, ep)
                                                      for i in range(4) for ep in range(8)]
                                        pending_fin = (pso, acc, it, qof + jc * 512, wbatch)
                            emit_fin(pending_fin)
                            emit_w(1 << 20)   # drain remaining W work
                        continue
                    # ---------------------------------------- phases A + W
                    with ExitStack() as SAW:
                        p1 = SAW.enter_context(tc.tile_pool(name=f"p1_{rep}", bufs=1))
                        attn_sb = p1.tile([128, QH * BT], BF16)
                        wo_sb = p1.tile([128, 4 * 4096], BF16)
                        nc.sync.dma_start(
                            out=wo_sb.rearrange("p (a m) -> p a m", a=4),
                            in_=wo_d.rearrange("a p m -> p a m"))

                        if "a" not in phases:   # "qw"/"wrep": attn from DRAM
                            nc.sync.dma_start(out=attn_sb, in_=attn_d)

                        def emit_A(irep):
                            # ---------------------------------- phase A
                            inv_d = nc.dram_tensor(
                                f"inv_i{rep}_{irep}", [B * QH * 4, 512], F32).ap()
                            with ExitStack() as SA:
                                prp = SA.enter_context(tc.tile_pool(name="prp", bufs=5))
                                acp = SA.enter_context(tc.tile_pool(name="acp", bufs=3))
                                ivp = SA.enter_context(tc.tile_pool(name="ivp", bufs=4))
                                pss_p = SA.enter_context(tc.tile_pool(name="pss", bufs=2, space="PSUM"))
                                pso_p = SA.enter_context(tc.tile_pool(name="pso", bufs=3, space="PSUM"))
                                psm_p = SA.enter_context(tc.tile_pool(name="psm", bufs=1, space="PSUM"))

                                def emit_fin(fin):
                                    # sums -> reciprocal -> broadcast -> normalize;
                                    # deferred one iteration so the chain latency
                                    # hides under the next iteration's matmuls
                                    pso, acc, it, col = fin
                                    psm = psm_p.tile([1, 512], F32, tag="m", name=f"psm_{it}")
                                    nc.tensor.matmul(psm, ones, acc[:, 0:512], start=True, stop=False)
                                    nc.tensor.matmul(psm, ones, acc[:, 512:1024], start=False, stop=True)
                                    inv_r = ivp.tile([1, 512], F32, tag="ivr", name=f"ivr_{it}")
                                    nc.vector.reciprocal(inv_r, psm)
                                    nc.sync.dma_start(out=inv_d[it:it + 1, :], in_=inv_r)
                                    inv_b = ivp.tile([128, 512], F32, tag="ivb", name=f"ivb_{it}")
                                    nc.sync.dma_start(
                                        out=inv_b, in_=inv_d[it:it + 1, :].to_broadcast((128, 512)))
                                    nc.vector.tensor_mul(attn_sb[:, col:col + 512], pso, inv_b)

                                pending_fin = None
                                for b in range(B):
                                    for h in range(QH):
                                        qof = h * BT + b * T
                                        for jc in range(4):
                                            it = (b * QH + h) * 4 + jc
                                            npair = 2 * (jc + 1)
                                            pso = pso_p.tile([128, 512], F32, tag="o", name=f"pso_{it}")
                                            acc = acp.tile([128, 1024], BF16, tag="acc", name=f"acc_{it}")

                                            def emit_pv(m, probs, pso=pso, b=b, npair=npair):
                                                for half in range(2):
                                                    tb = 2 * m + half
                                                    nc.tensor.matmul(
                                                        pso,
                                                        v_sb[:, (b * 16 + tb) * 128:(b * 16 + tb + 1) * 128],
                                                        probs[:, half * 512:(half + 1) * 512],
                                                        start=(tb == 0), stop=(tb == npair * 2 - 1))

                                            pend = None
                                            for m in range(npair):
                                                pss = pss_p.tile([128, 1024], F32, tag="s", name=f"pss_{it}_{m}")
                                                for half in range(2):
                                                    tb = 2 * m + half
                                                    nc.tensor.matmul(
                                                        pss[:, half * 512:(half + 1) * 512],
                                                        kt_sb[:, b * T + tb * 128:b * T + (tb + 1) * 128],
                                                        qt_sb[:, qof + jc * 512:qof + (jc + 1) * 512],
                                                        start=True, stop=True)
                                                probs = prp.tile([128, 1024], BF16, tag="pr", name=f"pr_{it}_{m}")
                                                nc.scalar.activation(probs, pss, EXP)
                                                for half in range(2):
                                                    o = 2 * m + half - 4 * jc
                                                    if o >= 0:
                                                        nc.vector.tensor_mul(
                                                            probs[:, half * 512:(half + 1) * 512],
                                                            probs[:, half * 512:(half + 1) * 512],
                                                            mk_sb[:, o * 512:(o + 1) * 512])
                                                if m == 0:
                                                    nc.vector.tensor_copy(acc, probs)
                                                else:
                                                    nc.vector.tensor_add(acc, acc, probs)
                                                # PV runs one m behind so exp latency
                                                # hides under the next scores matmuls
                                                if pend is not None:
                                                    emit_pv(m - 1, pend)
                                                pend = probs
                                                if m == 0 and pending_fin is not None:
                                                    emit_fin(pending_fin)
                                                    pending_fin = None
                                            emit_pv(npair - 1, pend)
                                            pending_fin = (pso, acc, it, qof + jc * 512)
                                emit_fin(pending_fin)

                        def emit_W(irep):
                            # ---------------------------------- phase W
                            with ExitStack() as SW:
                                orp = SW.enter_context(tc.tile_pool(name="orp", bufs=4))
                                psw_p = SW.enter_context(tc.tile_pool(name="psw", bufs=4, space="PSUM"))
                                for tb in range(BT // 128):
                                    for epp in range(4):
                                        ep0, ep1 = 2 * epp, 2 * epp + 1
                                        pws = [psw_p.tile([128, 512], F32, tag="w",
                                                          name=f"psw_{irep}_{tb}_{ep}")
                                               for ep in (ep0, ep1)]
                                        for db in range(4):
                                            for pi, ep in ((0, ep0), (1, ep1)):
                                                nc.tensor.matmul(
                                                    pws[pi],
                                                    attn_sb[:, db * BT + tb * 128:db * BT + (tb + 1) * 128],
                                                    wo_sb[:, db * 4096 + ep * 512:db * 4096 + (ep + 1) * 512],
                                                    start=(db == 0), stop=(db == 3))
                                        if "nocopy" in phases:
                                            continue
                                        for pi, ep in ((0, ep0), (1, ep1)):
                                            orow = orp.tile([128, 512], BF16, tag="or",
                                                            name=f"or_{irep}_{tb}_{ep}")
                                            nc.scalar.copy(orow, pws[pi])
                                            if "nodma" in phases:
                                                continue
                                            nc.sync.dma_start(out=out_d[tb * 8 + ep],
                                                              in_=orow)

                        for irep in range(inner_reps):
                            if "a" in phases:
                                emit_A(irep)
                            if phases in ("qa", "arep"):
                                continue
                            emit_W(irep)
    nc.finalize()
    return nc


def _host_prep(x, freqs, wq, wk, wv, wo, mask=None):
    if mask is None:
        ii = np.arange(T)[:, None]
        jj = np.arange(T)[None, :]
        mask = np.where(jj <= ii, np.float32(0.0), np.float32(-1e9))
    _kernel_mask = np.asarray(mask, dtype=np.float32)

    x = np.ascontiguousarray(np.asarray(x, dtype=np.float32).reshape(BT, D))
    xT = x.T.astype(NPBF)                                  # [D, BT] bf16
    # [TCH, 4, 128, 8*CW]: partition p of load (ch, q) reads one contiguous
    # 8KB line = rows {q*1024 + a*128 + p} of xT at chunk ch's columns
    xtc = np.ascontiguousarray(
        xT.reshape(4, 8, 128, TCH, CW).transpose(3, 0, 2, 1, 4)
        .reshape(TCH, 4, 128, 8 * CW))

    freqs = np.asarray(freqs, dtype=np.float32)
    cos = np.tile(np.cos(freqs).T, (1, B))                 # [64, BT]
    sin = np.tile(np.sin(freqs).T, (1, B))
    csa = np.concatenate([cos, cos], axis=0).astype(NPBF)
    csb = np.concatenate([-sin, sin], axis=0).astype(NPBF)
    swp = np.zeros((128, 128), np.float32)
    swp[np.arange(64), 64 + np.arange(64)] = 1.0
    swp[64 + np.arange(64), np.arange(64)] = 1.0

    perm = np.concatenate([np.arange(0, HD, 2), np.arange(1, HD, 2)])
    wq_p = (np.asarray(wq, dtype=np.float32).reshape(D, H, HD)[:, :, perm]
            .reshape(D, H * HD) / np.float32(np.sqrt(HD)))
    wk_p = np.asarray(wk, dtype=np.float32).reshape(D, HKV, HD)[:, :, perm].reshape(D, HKV * HD)
    wv = np.asarray(wv, dtype=np.float32)
    wo = np.asarray(wo, dtype=np.float32)

    # binary mask, transposed band layout:
    # maskt[:, o*512:(o+1)*512][i, j] = 1 if key (128*o+i) visible to query j
    maskt = np.concatenate(
        [(_kernel_mask[0:512, 128 * o:128 * o + 128] > -1.0).T.astype(np.float32)
         for o in range(4)],
        axis=1).astype(NPBF)                               # [128, 2048]
    ident = np.eye(128, dtype=np.float32).astype(NPBF)

    in_maps = []
    for c in range(NCORE):
        wq_c = np.ascontiguousarray(
            wq_p[:, c * 512:(c + 1) * 512].reshape(4, 8, 128, 512)
            .transpose(0, 2, 1, 3).reshape(4, 128, 8 * 512)).astype(NPBF)
        wk_c = np.ascontiguousarray(
            wk_p[:, c * HD:(c + 1) * HD].reshape(KBLK, 128, 128)
            .transpose(1, 0, 2).reshape(128, KBLK * 128)).astype(NPBF)
        wv_c = np.ascontiguousarray(
            wv[:, c * HD:(c + 1) * HD].reshape(KBLK, 128, 128)
            .transpose(1, 0, 2).reshape(128, KBLK * 128)).astype(NPBF)
        wo_c = np.ascontiguousarray(
            wo[c * 512:(c + 1) * 512, :].reshape(4, 128, 4096)).astype(NPBF)
        in_maps.append({
            "xt": xtc, "wqb": wq_c, "wkb": wk_c, "wvb": wv_c, "wob": wo_c,
            "csa": csa, "csb": csb, "maskt": maskt, "ident": ident,
            "swp": swp.astype(NPBF), "onesc": np.ones((128, 1), NPBF),
            "attn_in": np.zeros((128, QH * BT), NPBF),
        })
    return in_maps


def kernel(x, freqs, mask, wq, wk, wv, wo, start_pos=0, **_kw):
    global LAST_EXEC_NS
    in_maps = _host_prep(x, freqs, wq, wk, wv, wo, mask=mask)
    if "nc" not in _CACHE:
        _CACHE["nc"] = _build_nc(phases=PHASES)
    nc = _CACHE["nc"]
    res = run_bass_kernel_spmd(nc, in_maps, core_ids=list(range(NCORE)), trace=False)
    LAST_EXEC_NS = getattr(res, "exec_time_ns", None)
    total = res.results[0]["out"].astype(np.float32)
    for c in range(1, NCORE):
        total = total + res.results[c]["out"].astype(np.float32)
    # out[tb*8+ep, r, c] -> full[tb*128+r, ep*512+c]
    total = (total.reshape(32, 8, 128, 512).transpose(0, 2, 1, 3)
             .reshape(BT, D))
    return total.reshape(B, T, D)

